# revision 2
# baseline (speedup 1.0000x reference)
"""RNN-T JointNetwork kernel for 8 Trainium2 NeuronCores (raw bass), v3.

reference:
  e = enc @ W_enc.T + b_enc          # [B,T,H]
  d = dec @ W_dec.T + b_dec          # [B,U,H]
  j = tanh(e[:,:,None,:] + d[:,None,:,:])
  out = j @ W_joint.T + b_joint      # [B,T,U,V]

Sharding: T (256) split 8 ways -> 32 t-rows per core; host concatenates
along T.

v3 engine plan (vs v1's ACT-4-short-tanh + DVE-full-drain, which left the
kernel DVE/ACT-co-bound at ~1120ns/row vs PE's ~1048):

 - DVE+GPSIMD precompute X[h, row, u] = DT[h,u] + ET[h,row] via stride-0
   broadcast tensor_add, one instr per 4-row group per engine-share.
   This removes the per-(row,hi) bias constraint from ACT.
 - ACT computes tanh as ONE [128, 2048] instr per 4-row group (the
   ScalarE ~200cyc fixed SBUF-access cost amortizes 16x vs v1).
 - psum is a single [128, 4, VOCAB] tensor (4 row-slots x 2 banks).
   Drains are 2-row-pair instrs of full 1024 cols: even pairs (psum
   banks 0-3) on DVE tensor_copy, odd pairs (banks 4-7) on ACT Copy.
   The two drain engines always touch disjoint psum banks.
 - PE: per row, 8 bf16 matmuls [128x128]@[128x512] (4 k-tiles x 2 psum
   half-banks) - the critical engine at ~131ns/MM.

This toolchain's walrus rejects any compute instruction carrying >=2 sync
waits, so all cross-engine waits are standalone wait_ge instructions.
"""

import numpy as np

B, T, U = 4, 256, 128
ENC_DIM = DEC_DIM = HID = 512
VOCAB = 1024
NCORES = 8
TC = T // NCORES        # 32 t-rows per core
M = B * TC              # 128 (b,t) rows per core
HT = HID // 128         # 4 h tiles (contraction)
GR = 4                  # rows per tanh/X group
NG = M // GR            # 32 groups
NJR = 16                # jt ring, in rows (4 groups)
NXG = 4                 # X ring, in groups
NOT = 12                # output staging buffers (4-DMA-group release)
XD = 2                  # X-add h-tiles computed by DVE; (HT-XD) by GPSIMD

_CACHE = {}


def _build_bass(reps=1, timing=False):
    import concourse.bass as bass
    import concourse.mybir as mybir
    from concourse.bass import AP
    from concourse.ordered_set import OrderedSet

    f32 = mybir.dt.float32
    bf16 = mybir.dt.bfloat16
    Tanh = mybir.ActivationFunctionType.Tanh
    Copy = mybir.ActivationFunctionType.Copy

    nc = bass.Bass()
    ET = nc.declare_dram_parameter("ET", [128, HT, M], bf16, isOutput=False)
    DT = nc.declare_dram_parameter("DT", [128, HT, B * U], bf16, isOutput=False)
    WjT = nc.declare_dram_parameter("WjT", [128, HT, VOCAB], bf16, isOutput=False)
    if timing:
        out = nc.dram_tensor("out_i", [M, U, VOCAB], bf16)
        tok = nc.declare_dram_parameter("tok", [128, 4], bf16, isOutput=True)
    else:
        out = nc.declare_dram_parameter("out", [M, U, VOCAB], bf16, isOutput=True)

    from contextlib import ExitStack

    with ExitStack() as ctx:
        e = ctx.enter_context
        ET_sb = e(nc.sbuf_tensor("ET_sb", [128, HT, M], bf16))
        DT_sb = e(nc.sbuf_tensor("DT_sb", [128, HT, B * U], bf16))
        WjT_sb = e(nc.sbuf_tensor("WjT_sb", [128, HT, VOCAB], bf16))
        # X[g%NXG] = [GR rows, HT, 128u] contiguous 2048
        X_sb = e(nc.sbuf_tensor("X_sb", [128, NXG, GR, HT, 128], bf16))
        jt_sb = e(nc.sbuf_tensor("jt_sb", [128, NJR, HT, 128], bf16))
        ot_sb = e(nc.sbuf_tensor("ot_sb", [128, NOT, VOCAB], bf16))
        psu = e(nc.psum_tensor("psu", [128, 4, VOCAB], f32))
        s_in = e(nc.semaphore("s_in"))
        s_xd = e(nc.semaphore("s_xd"))    # DVE X groups done
        s_xg = e(nc.semaphore("s_xg"))    # GPSIMD X groups done
        s_act = e(nc.semaphore("s_act"))  # tanh groups done
        s_pe = e(nc.semaphore("s_pe"))    # matmul rows done
        s_dv = e(nc.semaphore("s_dv"))    # DVE (even) drain pairs done
        s_ac2 = e(nc.semaphore("s_ac2"))  # ACT (odd) drain pairs done
        s_outd = e(nc.semaphore("s_outd"))  # out-DMA 4-row groups (16 each)
        s_scr = [e(nc.semaphore(f"s_scr{i}")) for i in range(3)]
        block = e(nc.Block(no_gpsimd_drain=True))

        def hw_loop(eng, name, prologue_m, body_m, init_regs, n_iter,
                    epilogue=None):
            for m in range(n_iter):
                prologue_m(m)
            if reps <= 1:
                if epilogue is not None:
                    epilogue()
                return
            regs = {}
            for rn, iv in init_regs.items():
                r = eng.alloc_register(f"{name}_{rn}")
                eng.reg_mov(r, iv)
                regs[rn] = r
            r_loop = eng.alloc_register(f"{name}_loop")
            eng.reg_mov(r_loop, 0)
            top, end = f"{name}_top", f"{name}_end"
            eng.br_cmp(r_loop, reps - 1, top, end, "IS_LT")
            with nc.body(top, valid_engines=OrderedSet([eng.engine])):
                for m in range(n_iter):
                    body_m(m, regs)
                eng.reg_add(r_loop, r_loop, 1)
                eng.br_cmp(r_loop, reps - 1, top, end, "IS_LT")
            with nc.body(end, valid_engines=OrderedSet([eng.engine])):
                if epilogue is not None:
                    epilogue()
                else:
                    eng.nop()
            block.last_body[eng] = end

        # ---- X-add helper: group g, h-tiles [h0, h1) on engine `eng` ----
        def xadd(eng, g, h0, h1):
            nh = h1 - h0
            b = (GR * g) // TC
            xa = X_sb[:, g % NXG, :, :, :]
            out_ap = AP(
                xa.tensor,
                xa.offset + h0 * 128,
                [xa.ap[0], (HT * 128, GR), (128, nh), (1, 128)],
            )
            a0 = DT_sb[:, :, :]
            in0 = AP(
                a0.tensor,
                a0.offset + h0 * (B * U) + b * 128,
                [a0.ap[0], (0, GR), (B * U, nh), (1, 128)],
            )
            a1 = ET_sb[:, :, :]
            in1 = AP(
                a1.tensor,
                a1.offset + h0 * M + GR * g,
                [a1.ap[0], (1, GR), (M, nh), (0, 128)],
            )
            return eng.tensor_add(out_ap, in0, in1)

        # ---- drain helper: pair q (rows 2q, 2q+1) ----------------------
        def drain(eng, q, is_act):
            s = (2 * q) % 4
            o = (2 * q) % NOT
            src = psu[:, s : s + 2, :]
            dst = ot_sb[:, o : o + 2, :]
            if is_act:
                return eng.activation(dst, src, Copy)
            return eng.tensor_copy(dst, src)

        # --- SP: input DMAs, then per-row output DMAs -------------------
        @block.sync
        def _(sync):
            sync.dma_start(out=ET_sb[:], in_=ET[:]).then_inc(s_in, 16)
            sync.dma_start(out=DT_sb[:], in_=DT[:]).then_inc(s_in, 16)
            for hi in range(HT):
                sync.dma_start(
                    out=WjT_sb[:, hi, :], in_=WjT[:, hi, :]
                ).then_inc(s_in, 16)
            if timing:
                sync.dma_start(out=tok[:], in_=ET_sb[:, 0, 0:4]).then_inc(
                    s_in, 16
                )

            def dma_row(r):
                d = sync.dma_start(out=out[r], in_=ot_sb[:, r % NOT, :])
                if r % 4 == 3:
                    d.then_inc(s_outd, 16)
                else:
                    d.then_inc(s_scr[r % 3], 16)

            def pro(g):
                for j in range(GR):
                    r = GR * g + j
                    if j == 0:
                        sync.wait_ge(s_dv, g + 1)
                    elif j == 2:
                        sync.wait_ge(s_ac2, g + 1)
                    dma_row(r)

            def body(g, regs):
                for j in range(GR):
                    r = GR * g + j
                    if j == 0:
                        sync.wait_ge(s_dv, regs["dv"])
                        sync.reg_add(regs["dv"], regs["dv"], 1)
                    elif j == 2:
                        sync.wait_ge(s_ac2, regs["ac2"])
                        sync.reg_add(regs["ac2"], regs["ac2"], 1)
                    dma_row(r)

            hw_loop(
                sync, "sp", pro, body, {"dv": NG + 1, "ac2": NG + 1}, NG,
                epilogue=lambda: sync.wait_ge(s_outd, 4 * M * reps),
            )

        # --- PE: 8 matmuls per row --------------------------------------
        @block.tensor
        def _(pe):
            def mms(m):
                for hi in range(HT):
                    for vi in range(2):
                        mm = pe.matmul(
                            psu[:, m % 4, vi * 512 : (vi + 1) * 512],
                            jt_sb[:, m % NJR, hi, :],
                            WjT_sb[:, hi, vi * 512 : (vi + 1) * 512],
                            start=(hi == 0),
                            stop=(hi == HT - 1),
                        )
                return mm

            def pro(g):
                for j in range(GR):
                    m = GR * g + j
                    if j == 0:
                        pe.wait_ge(s_act, g + 1)
                    if m == 0:
                        pe.wait_ge(s_in, 16 * 6)  # ET+DT+all WjT chunks
                    if m >= 4:
                        # psum slots {m%4,(m+1)%4} freed by pair (m-4)/2
                        if j == 0:
                            pe.wait_ge(s_dv, m // 4)
                        elif j == 2:
                            pe.wait_ge(s_ac2, (m - 2) // 4)
                    mms(m).then_inc(s_pe, 1)

            def body(g, regs):
                for j in range(GR):
                    m = GR * g + j
                    if j == 0:
                        pe.wait_ge(s_act, regs["act"])
                        pe.reg_add(regs["act"], regs["act"], 1)
                        pe.wait_ge(s_dv, regs["dv"])
                        pe.reg_add(regs["dv"], regs["dv"], 1)
                    elif j == 2:
                        pe.wait_ge(s_ac2, regs["ac2"])
                        pe.reg_add(regs["ac2"], regs["ac2"], 1)
                    mms(m).then_inc(s_pe, 1)

            hw_loop(
                pe, "pe", pro, body,
                {"act": NG + 1, "dv": M // 4, "ac2": M // 4},
                NG,
            )

        # --- ACT: tanh per group + odd-pair drains ----------------------
        @block.scalar
        def _(act):
            def tanh_g(g):
                act.activation(
                    jt_sb[:, (GR * g) % NJR : (GR * g) % NJR + GR, :, :],
                    X_sb[:, g % NXG, :, :, :],
                    Tanh,
                ).then_inc(s_act, 1)

            def pro(g):
                act.wait_ge(s_xd, g + 1)
                act.wait_ge(s_xg, g + 1)
                if g >= NJR // GR:
                    act.wait_ge(s_pe, GR * g - NJR + GR)
                tanh_g(g)
                if g >= 1:
                    q = 2 * g - 1
                    if 2 * q >= NOT:
                        act.wait_ge(s_outd, 16 * ((2 * q - NOT) // 4 + 1))
                    act.wait_ge(s_pe, 2 * q + 2)
                    drain(act, q, True).then_inc(s_ac2, 1)

            def body(g, regs):
                act.wait_ge(s_xd, regs["xd"])
                act.reg_add(regs["xd"], regs["xd"], 1)
                act.wait_ge(s_xg, regs["xg"])
                act.reg_add(regs["xg"], regs["xg"], 1)
                act.wait_ge(s_pe, regs["pe_t"])
                act.reg_add(regs["pe_t"], regs["pe_t"], GR)
                tanh_g(g)
                q = (2 * g - 1) % (M // 2)
                act.wait_ge(s_outd, regs["outd"])
                act.reg_add(regs["outd"], regs["outd"], 16)
                act.wait_ge(s_pe, regs["pe_d"])
                act.reg_add(regs["pe_d"], regs["pe_d"], GR)
                drain(act, q, True).then_inc(s_ac2, 1)

            def epi():
                q = M // 2 - 1
                act.wait_ge(s_outd, 16 * (32 * reps - 3))
                act.wait_ge(s_pe, M * reps)
                drain(act, q, True).then_inc(s_ac2, 1)

            hw_loop(
                act, "act", pro, body,
                {
                    "xd": NG + 1,
                    "xg": NG + 1,
                    "pe_t": GR * NG - NJR + GR,
                    "outd": 16 * ((2 * (2 * NG - 1) - NOT) // 4 + 1),
                    "pe_d": M,
                },
                NG,
                epilogue=epi,
            )

        # --- DVE: X-add share + even-pair drains ------------------------
        @block.vector
        def _(dve):
            def pro(g):
                if g == 0:
                    dve.wait_ge(s_in, 32)
                    for gg in range(min(3, NG)):
                        xadd(dve, gg, 0, XD).then_inc(s_xd, 1)
                else:
                    gx = g + 2
                    if gx < NG:
                        if gx >= NXG:
                            dve.wait_ge(s_act, gx - NXG + 1)
                        xadd(dve, gx, 0, XD).then_inc(s_xd, 1)
                    q = 2 * (g - 1)
                    if 2 * q >= NOT:
                        dve.wait_ge(s_outd, 16 * ((2 * q - NOT) // 4 + 1))
                    dve.wait_ge(s_pe, 2 * q + 2)
                    drain(dve, q, False).then_inc(s_dv, 1)

            def body(g, regs):
                gx = (g + 2) % NG
                dve.wait_ge(s_act, regs["act"])
                dve.reg_add(regs["act"], regs["act"], 1)
                xadd(dve, gx, 0, XD).then_inc(s_xd, 1)
                q = (2 * (g - 1)) % (M // 2)
                dve.wait_ge(s_outd, regs["outd"])
                dve.reg_add(regs["outd"], regs["outd"], 16)
                dve.wait_ge(s_pe, regs["pe"])
                dve.reg_add(regs["pe"], regs["pe"], GR)
                drain(dve, q, False).then_inc(s_dv, 1)

            def epi():
                q = M // 2 - 2
                dve.wait_ge(s_outd, 16 * (32 * reps - 3))
                dve.wait_ge(s_pe, M * reps)
                drain(dve, q, False).then_inc(s_dv, 1)

            hw_loop(
                dve, "dve", pro, body,
                {
                    "act": NG + 2 - NXG + 1,
                    "outd": 16 * ((2 * 2 * (NG - 1) - NOT) // 4 + 1),
                    "pe": M - 2,
                },
                NG,
                epilogue=epi,
            )

        # --- GPSIMD: remaining X-add h-tiles ----------------------------
        @block.gpsimd
        def _(gp):
            def pro(g):
                if g == 0:
                    gp.wait_ge(s_in, 32)
                    for gg in range(min(3, NG)):
                        xadd(gp, gg, XD, HT).then_inc(s_xg, 1)
                else:
                    gx = g + 2
                    if gx < NG:
                        if gx >= NXG:
                            gp.wait_ge(s_act, gx - NXG + 1)
                        xadd(gp, gx, XD, HT).then_inc(s_xg, 1)
                    else:
                        gp.nop()

            def body(g, regs):
                gx = (g + 2) % NG
                gp.wait_ge(s_act, regs["act"])
                gp.reg_add(regs["act"], regs["act"], 1)
                xadd(gp, gx, XD, HT).then_inc(s_xg, 1)

            hw_loop(
                gp, "gp", pro, body,
                {"act": NG + 2 - NXG + 1},
                NG,
            )

    return nc


def _tile_h(a, dtype):
    """[N, H] -> [128, H//128, N] with h = hi*128 + p."""
    n, h = a.shape
    return np.ascontiguousarray(
        a.reshape(n, h // 128, 128).transpose(2, 1, 0).astype(dtype)
    )


def _prep_inputs(enc_out, dec_out, W_enc, b_enc, W_dec, b_dec, W_joint, b_joint):
    import ml_dtypes

    bf16 = ml_dtypes.bfloat16
    enc_out = np.asarray(enc_out, dtype=np.float32)
    dec_out = np.asarray(dec_out, dtype=np.float32)
    W_enc = np.asarray(W_enc, np.float32)
    W_dec = np.asarray(W_dec, np.float32)
    W_joint = np.asarray(W_joint, np.float32)
    b_enc = np.asarray(b_enc, np.float32)
    b_dec = np.asarray(b_dec, np.float32)

    # host-side small projections (0.3% of total FLOPs), fp32
    e_full = enc_out.reshape(B * T, ENC_DIM) @ W_enc.T + b_enc  # [B*T, H]
    d_full = dec_out.reshape(B * U, DEC_DIM) @ W_dec.T + b_dec  # [B*U, H]
    e_full = e_full.reshape(B, T, HID)

    common = {
        "DT": _tile_h(d_full, bf16),
        "WjT": _tile_h(np.ascontiguousarray(W_joint), bf16),
    }
    in_maps = []
    for i in range(NCORES):
        sl = e_full[:, i * TC : (i + 1) * TC, :].reshape(M, HID)
        m = dict(common)
        m["ET"] = _tile_h(sl, bf16)
        in_maps.append(m)
    return in_maps


def run(in_maps, trace=False, **kw):
    from concourse.bass_utils import run_bass_kernel_spmd

    if "nc" not in _CACHE:
        _CACHE["nc"] = _build_bass()
    return run_bass_kernel_spmd(
        _CACHE["nc"], in_maps, list(range(NCORES)), trace=trace, **kw
    )


def time_kernel(in_maps, reps_list=(2, 2002), n_meas=6):
    """HW time per main-loop pass via rep-count wall-clock deltas."""
    import time
    from concourse.bass_utils import run_bass_kernel_spmd

    walls = {}
    for reps in reps_list:
        key = f"t{reps}"
        if key not in _CACHE:
            _CACHE[key] = _build_bass(reps=reps, timing=True)
        nc = _CACHE[key]
        run_bass_kernel_spmd(nc, in_maps, list(range(NCORES)))  # compile+warm
        ts = []
        for _ in range(n_meas):
            t0 = time.time()
            run_bass_kernel_spmd(nc, in_maps, list(range(NCORES)))
            ts.append(time.time() - t0)
        walls[reps] = min(ts)
    r0, r1 = reps_list
    per_pass = (walls[r1] - walls[r0]) / (r1 - r0)
    return per_pass, walls


def kernel(enc_out, dec_out, W_enc, b_enc, W_dec, b_dec, W_joint, b_joint):
    import sys

    if "/opt/trn_rl_repo" not in sys.path:
        sys.path.insert(0, "/opt/trn_rl_repo")

    in_maps = _prep_inputs(
        enc_out, dec_out, W_enc, b_enc, W_dec, b_dec, W_joint, b_joint
    )
    res = run(in_maps)
    bj = np.asarray(b_joint, np.float32)
    parts = [
        r["out"].astype(np.float32).reshape(B, TC, U, VOCAB) for r in res.results
    ]
    return np.concatenate(parts, axis=1) + bj


# revision 3
# speedup vs baseline: 1.0425x; 1.0425x over previous
"""RNN-T JointNetwork kernel for 8 Trainium2 NeuronCores (raw bass), v3.

reference:
  e = enc @ W_enc.T + b_enc          # [B,T,H]
  d = dec @ W_dec.T + b_dec          # [B,U,H]
  j = tanh(e[:,:,None,:] + d[:,None,:,:])
  out = j @ W_joint.T + b_joint      # [B,T,U,V]

Sharding: T (256) split 8 ways -> 32 t-rows per core; host concatenates
along T.

v3 engine plan (vs v1's ACT-4-short-tanh + DVE-full-drain, which left the
kernel DVE/ACT-co-bound at ~1120ns/row vs PE's ~1048):

 - DVE+GPSIMD precompute X[h, row, u] = DT[h,u] + ET[h,row] via stride-0
   broadcast tensor_add, one instr per 4-row group per engine-share.
   This removes the per-(row,hi) bias constraint from ACT.
 - ACT computes tanh as ONE [128, 2048] instr per 4-row group (the
   ScalarE ~200cyc fixed SBUF-access cost amortizes 16x vs v1).
 - psum is a single [128, 4, VOCAB] tensor (4 row-slots x 2 banks).
   Drains are 2-row-pair instrs of full 1024 cols: even pairs (psum
   banks 0-3) on DVE tensor_copy, odd pairs (banks 4-7) on ACT Copy.
   The two drain engines always touch disjoint psum banks.
 - PE: per row, 8 bf16 matmuls [128x128]@[128x512] (4 k-tiles x 2 psum
   half-banks) - the critical engine at ~131ns/MM.

This toolchain's walrus rejects any compute instruction carrying >=2 sync
waits, so all cross-engine waits are standalone wait_ge instructions.
"""

import numpy as np

B, T, U = 4, 256, 128
ENC_DIM = DEC_DIM = HID = 512
VOCAB = 1024
NCORES = 8
TC = T // NCORES        # 32 t-rows per core
M = B * TC              # 128 (b,t) rows per core
HT = HID // 128         # 4 h tiles (contraction)
GR = 4                  # rows per tanh/X group
NG = M // GR            # 32 groups
NJR = 16                # jt ring, in rows (4 groups)
NXG = 4                 # X ring, in groups
NOT = 12                # output staging buffers (4-DMA-group release)
XD = 2                  # X-add h-tiles computed by DVE; (HT-XD) by GPSIMD

_CACHE = {}


def _build_bass(reps=1, timing=False):
    import concourse.bass as bass
    import concourse.mybir as mybir
    from concourse.bass import AP
    from concourse.ordered_set import OrderedSet

    f32 = mybir.dt.float32
    bf16 = mybir.dt.bfloat16
    Tanh = mybir.ActivationFunctionType.Tanh
    Copy = mybir.ActivationFunctionType.Copy

    nc = bass.Bass()
    ET = nc.declare_dram_parameter("ET", [128, HT, M], bf16, isOutput=False)
    DT = nc.declare_dram_parameter("DT", [128, HT, B * U], bf16, isOutput=False)
    WjT = nc.declare_dram_parameter("WjT", [128, HT, VOCAB], bf16, isOutput=False)
    if timing:
        out = nc.dram_tensor("out_i", [M, U, VOCAB], bf16)
        tok = nc.declare_dram_parameter("tok", [128, 4], bf16, isOutput=True)
    else:
        out = nc.declare_dram_parameter("out", [M, U, VOCAB], bf16, isOutput=True)

    from contextlib import ExitStack

    with ExitStack() as ctx:
        e = ctx.enter_context
        ET_sb = e(nc.sbuf_tensor("ET_sb", [128, HT, M], bf16))
        DT_sb = e(nc.sbuf_tensor("DT_sb", [128, HT, B * U], bf16))
        WjT_sb = e(nc.sbuf_tensor("WjT_sb", [128, HT, VOCAB], bf16))
        # X[g%NXG] = [GR rows, HT, 128u] contiguous 2048
        X_sb = e(nc.sbuf_tensor("X_sb", [128, NXG, GR, HT, 128], bf16))
        jt_sb = e(nc.sbuf_tensor("jt_sb", [128, NJR, HT, 128], bf16))
        ot_sb = e(nc.sbuf_tensor("ot_sb", [128, NOT, VOCAB], bf16))
        psu = e(nc.psum_tensor("psu", [128, 4, VOCAB], f32))
        s_in = e(nc.semaphore("s_in"))
        s_xd = e(nc.semaphore("s_xd"))    # DVE X groups done
        s_xg = e(nc.semaphore("s_xg"))    # GPSIMD X groups done
        s_act = e(nc.semaphore("s_act"))  # tanh groups done
        s_pe = e(nc.semaphore("s_pe"))    # matmul rows done
        s_dv = e(nc.semaphore("s_dv"))    # DVE (even) drain pairs done
        s_ac2 = e(nc.semaphore("s_ac2"))  # ACT (odd) drain pairs done
        s_outd = e(nc.semaphore("s_outd"))  # out-DMA 4-row groups (16 each)
        s_scr = [e(nc.semaphore(f"s_scr{i}")) for i in range(3)]
        block = e(nc.Block(no_gpsimd_drain=True))

        def hw_loop(eng, name, prologue_m, body_m, init_regs, n_iter,
                    epilogue=None):
            for m in range(n_iter):
                prologue_m(m)
            if reps <= 1:
                if epilogue is not None:
                    epilogue()
                return
            regs = {}
            for rn, iv in init_regs.items():
                r = eng.alloc_register(f"{name}_{rn}")
                eng.reg_mov(r, iv)
                regs[rn] = r
            r_loop = eng.alloc_register(f"{name}_loop")
            eng.reg_mov(r_loop, 0)
            top, end = f"{name}_top", f"{name}_end"
            eng.br_cmp(r_loop, reps - 1, top, end, "IS_LT")
            with nc.body(top, valid_engines=OrderedSet([eng.engine])):
                for m in range(n_iter):
                    body_m(m, regs)
                eng.reg_add(r_loop, r_loop, 1)
                eng.br_cmp(r_loop, reps - 1, top, end, "IS_LT")
            with nc.body(end, valid_engines=OrderedSet([eng.engine])):
                if epilogue is not None:
                    epilogue()
                else:
                    eng.nop()
            block.last_body[eng] = end

        # ---- X-add helper: group g, h-tiles [h0, h1) on engine `eng` ----
        def xadd(eng, g, h0, h1):
            nh = h1 - h0
            b = (GR * g) // TC
            xa = X_sb[:, g % NXG, :, :, :]
            out_ap = AP(
                xa.tensor,
                xa.offset + h0 * 128,
                [xa.ap[0], (HT * 128, GR), (128, nh), (1, 128)],
            )
            a0 = DT_sb[:, :, :]
            in0 = AP(
                a0.tensor,
                a0.offset + h0 * (B * U) + b * 128,
                [a0.ap[0], (0, GR), (B * U, nh), (1, 128)],
            )
            a1 = ET_sb[:, :, :]
            in1 = AP(
                a1.tensor,
                a1.offset + h0 * M + GR * g,
                [a1.ap[0], (1, GR), (M, nh), (0, 128)],
            )
            return eng.tensor_add(out_ap, in0, in1)

        # ---- drain helper: pair q (rows 2q, 2q+1) ----------------------
        def drain(eng, q, is_act):
            s = (2 * q) % 4
            o = (2 * q) % NOT
            src = psu[:, s : s + 2, :]
            dst = ot_sb[:, o : o + 2, :]
            if is_act:
                return eng.activation(dst, src, Copy)
            return eng.tensor_copy(dst, src)

        # --- SP: input DMAs, then per-row output DMAs -------------------
        @block.sync
        def _(sync):
            sync.dma_start(out=ET_sb[:], in_=ET[:]).then_inc(s_in, 16)
            sync.dma_start(out=DT_sb[:], in_=DT[:]).then_inc(s_in, 16)
            for hi in range(HT):
                sync.dma_start(
                    out=WjT_sb[:, hi, :], in_=WjT[:, hi, :]
                ).then_inc(s_in, 16)
            if timing:
                sync.dma_start(out=tok[:], in_=ET_sb[:, 0, 0:4]).then_inc(
                    s_in, 16
                )

            def dma_row(r):
                d = sync.dma_start(out=out[r], in_=ot_sb[:, r % NOT, :])
                if r % 4 == 3:
                    d.then_inc(s_outd, 16)
                else:
                    d.then_inc(s_scr[r % 3], 16)

            def pro(g):
                for j in range(GR):
                    r = GR * g + j
                    if j == 0:
                        sync.wait_ge(s_dv, 2 * g + 2)
                    elif j == 2:
                        sync.wait_ge(s_ac2, g + 1)
                    dma_row(r)

            def body(g, regs):
                for j in range(GR):
                    r = GR * g + j
                    if j == 0:
                        sync.wait_ge(s_dv, regs["dv"])
                        sync.reg_add(regs["dv"], regs["dv"], 2)
                    elif j == 2:
                        sync.wait_ge(s_ac2, regs["ac2"])
                        sync.reg_add(regs["ac2"], regs["ac2"], 1)
                    dma_row(r)

            hw_loop(
                sync, "sp", pro, body, {"dv": NG + 1, "ac2": NG + 1}, NG,
                epilogue=lambda: sync.wait_ge(s_outd, 4 * M * reps),
            )

        # --- PE: 8 matmuls per row --------------------------------------
        @block.tensor
        def _(pe):
            def mms(m):
                for hi in range(HT):
                    for vi in range(2):
                        mm = pe.matmul(
                            psu[:, m % 4, vi * 512 : (vi + 1) * 512],
                            jt_sb[:, m % NJR, hi, :],
                            WjT_sb[:, hi, vi * 512 : (vi + 1) * 512],
                            start=(hi == 0),
                            stop=(hi == HT - 1),
                        )
                return mm

            def pro(g):
                for j in range(GR):
                    m = GR * g + j
                    if j == 0:
                        pe.wait_ge(s_act, g + 1)
                    if m == 0:
                        pe.wait_ge(s_in, 16 * 6)  # ET+DT+all WjT chunks
                    if m >= 4:
                        # psum slots {m%4,(m+1)%4} freed by pair (m-4)/2
                        if j == 0:
                            pe.wait_ge(s_dv, (m - 4) // 2 + 2)
                        elif j == 2:
                            pe.wait_ge(s_ac2, (m - 2) // 4)
                    mms(m).then_inc(s_pe, 1)

            def body(g, regs):
                for j in range(GR):
                    m = GR * g + j
                    if j == 0:
                        pe.wait_ge(s_act, regs["act"])
                        pe.reg_add(regs["act"], regs["act"], 1)
                        pe.wait_ge(s_dv, regs["dv"])
                        pe.reg_add(regs["dv"], regs["dv"], 2)
                    elif j == 2:
                        pe.wait_ge(s_ac2, regs["ac2"])
                        pe.reg_add(regs["ac2"], regs["ac2"], 1)
                    mms(m).then_inc(s_pe, 1)

            hw_loop(
                pe, "pe", pro, body,
                {"act": NG + 1, "dv": M // 2, "ac2": M // 4},
                NG,
            )

        # --- ACT: tanh per group + odd-pair drains ----------------------
        @block.scalar
        def _(act):
            def tanh_g(g):
                act.activation(
                    jt_sb[:, (GR * g) % NJR : (GR * g) % NJR + GR, :, :],
                    X_sb[:, g % NXG, :, :, :],
                    Tanh,
                ).then_inc(s_act, 1)

            def pro(g):
                act.wait_ge(s_xd, g + 1)
                act.wait_ge(s_xg, g + 1)
                if g >= NJR // GR:
                    act.wait_ge(s_pe, GR * g - NJR + GR)
                tanh_g(g)
                if g >= 1:
                    q = 2 * g - 1
                    if 2 * q >= NOT:
                        act.wait_ge(s_outd, 16 * ((2 * q - NOT) // 4 + 1))
                    act.wait_ge(s_pe, 2 * q + 2)
                    drain(act, q, True).then_inc(s_ac2, 1)

            def body(g, regs):
                act.wait_ge(s_xd, regs["xd"])
                act.reg_add(regs["xd"], regs["xd"], 1)
                act.wait_ge(s_xg, regs["xg"])
                act.reg_add(regs["xg"], regs["xg"], 1)
                act.wait_ge(s_pe, regs["pe_t"])
                act.reg_add(regs["pe_t"], regs["pe_t"], GR)
                tanh_g(g)
                q = (2 * g - 1) % (M // 2)
                act.wait_ge(s_outd, regs["outd"])
                act.reg_add(regs["outd"], regs["outd"], 16)
                act.wait_ge(s_pe, regs["pe_d"])
                act.reg_add(regs["pe_d"], regs["pe_d"], GR)
                drain(act, q, True).then_inc(s_ac2, 1)

            def epi():
                q = M // 2 - 1
                act.wait_ge(s_outd, 16 * (32 * reps - 3))
                act.wait_ge(s_pe, M * reps)
                drain(act, q, True).then_inc(s_ac2, 1)

            hw_loop(
                act, "act", pro, body,
                {
                    "xd": NG + 1,
                    "xg": NG + 1,
                    "pe_t": GR * NG - NJR + GR,
                    "outd": 16 * ((2 * (2 * NG - 1) - NOT) // 4 + 1),
                    "pe_d": M,
                },
                NG,
                epilogue=epi,
            )

        # --- DVE: X-add share + even-pair drains (one instr per row, so
        # the first row's drain starts as soon as PE finishes that row) ---
        def drain_row(eng, r):
            return eng.tensor_copy(ot_sb[:, r % NOT, :], psu[:, r % 4, :])

        @block.vector
        def _(dve):
            def pro(g):
                if g == 0:
                    dve.wait_ge(s_in, 32)
                    for gg in range(min(3, NG)):
                        xadd(dve, gg, 0, XD).then_inc(s_xd, 1)
                else:
                    gx = g + 2
                    if gx < NG:
                        if gx >= NXG:
                            dve.wait_ge(s_act, gx - NXG + 1)
                        xadd(dve, gx, 0, XD).then_inc(s_xd, 1)
                    r0 = 4 * (g - 1)
                    if r0 >= NOT:
                        dve.wait_ge(s_outd, 16 * ((r0 - NOT) // 4 + 1))
                    dve.wait_ge(s_pe, r0 + 1)
                    drain_row(dve, r0).then_inc(s_dv, 1)
                    dve.wait_ge(s_pe, r0 + 2)
                    drain_row(dve, r0 + 1).then_inc(s_dv, 1)

            def body(g, regs):
                gx = (g + 2) % NG
                dve.wait_ge(s_act, regs["act"])
                dve.reg_add(regs["act"], regs["act"], 1)
                xadd(dve, gx, 0, XD).then_inc(s_xd, 1)
                r0 = (4 * (g - 1)) % M
                dve.wait_ge(s_outd, regs["outd"])
                dve.reg_add(regs["outd"], regs["outd"], 16)
                dve.wait_ge(s_pe, regs["pe"])
                dve.reg_add(regs["pe"], regs["pe"], 1)
                drain_row(dve, r0).then_inc(s_dv, 1)
                dve.wait_ge(s_pe, regs["pe"])
                dve.reg_add(regs["pe"], regs["pe"], 3)
                drain_row(dve, r0 + 1).then_inc(s_dv, 1)

            def epi():
                dve.wait_ge(s_outd, 16 * (32 * reps - 3))
                dve.wait_ge(s_pe, M * reps)
                drain_row(dve, M - 4).then_inc(s_dv, 1)
                drain_row(dve, M - 3).then_inc(s_dv, 1)

            hw_loop(
                dve, "dve", pro, body,
                {
                    "act": NG + 2 - NXG + 1,
                    "outd": 16 * ((4 * (NG - 1) - NOT) // 4 + 1),
                    "pe": M - 3,
                },
                NG,
                epilogue=epi,
            )

        # --- GPSIMD: remaining X-add h-tiles ----------------------------
        @block.gpsimd
        def _(gp):
            def pro(g):
                if g == 0:
                    gp.wait_ge(s_in, 32)
                    for gg in range(min(3, NG)):
                        xadd(gp, gg, XD, HT).then_inc(s_xg, 1)
                else:
                    gx = g + 2
                    if gx < NG:
                        if gx >= NXG:
                            gp.wait_ge(s_act, gx - NXG + 1)
                        xadd(gp, gx, XD, HT).then_inc(s_xg, 1)
                    else:
                        gp.nop()

            def body(g, regs):
                gx = (g + 2) % NG
                gp.wait_ge(s_act, regs["act"])
                gp.reg_add(regs["act"], regs["act"], 1)
                xadd(gp, gx, XD, HT).then_inc(s_xg, 1)

            hw_loop(
                gp, "gp", pro, body,
                {"act": NG + 2 - NXG + 1},
                NG,
            )

    return nc


def _tile_h(a, dtype):
    """[N, H] -> [128, H//128, N] with h = hi*128 + p."""
    n, h = a.shape
    return np.ascontiguousarray(
        a.reshape(n, h // 128, 128).transpose(2, 1, 0).astype(dtype)
    )


def _prep_inputs(enc_out, dec_out, W_enc, b_enc, W_dec, b_dec, W_joint, b_joint):
    import ml_dtypes

    bf16 = ml_dtypes.bfloat16
    enc_out = np.asarray(enc_out, dtype=np.float32)
    dec_out = np.asarray(dec_out, dtype=np.float32)
    W_enc = np.asarray(W_enc, np.float32)
    W_dec = np.asarray(W_dec, np.float32)
    W_joint = np.asarray(W_joint, np.float32)
    b_enc = np.asarray(b_enc, np.float32)
    b_dec = np.asarray(b_dec, np.float32)

    # host-side small projections (0.3% of total FLOPs), fp32
    e_full = enc_out.reshape(B * T, ENC_DIM) @ W_enc.T + b_enc  # [B*T, H]
    d_full = dec_out.reshape(B * U, DEC_DIM) @ W_dec.T + b_dec  # [B*U, H]
    e_full = e_full.reshape(B, T, HID)

    common = {
        "DT": _tile_h(d_full, bf16),
        "WjT": _tile_h(np.ascontiguousarray(W_joint), bf16),
    }
    in_maps = []
    for i in range(NCORES):
        sl = e_full[:, i * TC : (i + 1) * TC, :].reshape(M, HID)
        m = dict(common)
        m["ET"] = _tile_h(sl, bf16)
        in_maps.append(m)
    return in_maps


def run(in_maps, trace=False, **kw):
    from concourse.bass_utils import run_bass_kernel_spmd

    if "nc" not in _CACHE:
        _CACHE["nc"] = _build_bass()
    return run_bass_kernel_spmd(
        _CACHE["nc"], in_maps, list(range(NCORES)), trace=trace, **kw
    )


def time_kernel(in_maps, reps_list=(2, 2002), n_meas=6):
    """HW time per main-loop pass via rep-count wall-clock deltas."""
    import time
    from concourse.bass_utils import run_bass_kernel_spmd

    walls = {}
    for reps in reps_list:
        key = f"t{reps}"
        if key not in _CACHE:
            _CACHE[key] = _build_bass(reps=reps, timing=True)
        nc = _CACHE[key]
        run_bass_kernel_spmd(nc, in_maps, list(range(NCORES)))  # compile+warm
        ts = []
        for _ in range(n_meas):
            t0 = time.time()
            run_bass_kernel_spmd(nc, in_maps, list(range(NCORES)))
            ts.append(time.time() - t0)
        walls[reps] = min(ts)
    r0, r1 = reps_list
    per_pass = (walls[r1] - walls[r0]) / (r1 - r0)
    return per_pass, walls


def kernel(enc_out, dec_out, W_enc, b_enc, W_dec, b_dec, W_joint, b_joint):
    import sys

    if "/opt/trn_rl_repo" not in sys.path:
        sys.path.insert(0, "/opt/trn_rl_repo")

    in_maps = _prep_inputs(
        enc_out, dec_out, W_enc, b_enc, W_dec, b_dec, W_joint, b_joint
    )
    res = run(in_maps)
    bj = np.asarray(b_joint, np.float32)
    parts = [
        r["out"].astype(np.float32).reshape(B, TC, U, VOCAB) for r in res.results
    ]
    return np.concatenate(parts, axis=1) + bj
